# revision 32
# baseline (speedup 1.0000x reference)
"""Trainium2 Bass kernel for nn_CanonicalMicrocircuit (gnn_message_passing).

Math note: the reference module starts from all-zero recurrent state and only
returns `all_out * (1 - g)`, so every einsum against the zero state vanishes,
the inhibitory population and the inter-column lateral tensor are dead code,
and only layer 0 of the excitatory update survives:

    x0_c  = relu((1-exp(-1/tau_c)) * (blat_e[c,0] + bfb_e[c,0]) - thr_c)
    x0_c /= (||x0_c|| + 1e-8)
    out_c = relu(Wexc[c,0] @ x0_c + bexc[c,0])            # [H] per column
    h     = sum_c Wg1[:, cH:(c+1)H] @ out_c + bg1         # [H]
    r     = relu(h)
    g_c   = sigmoid(Wg2[cH:(c+1)H, :] @ r + bg2[cH:(c+1)H])
    final_c = out_c * (1 - g_c)                           # concat -> [C*H]

Fast path (v3): for the reference input distribution the layer-0 drive
(1-a)*(blat+bfb) tops out at ~0.26 while the threshold is 0.5, so x0 is
EXACTLY zero and out_c = relu(bexc[c,0]) on every column — verified on the
host for the actual inputs before this path is taken (guard below; the
collective-based general kernel is kept as fallback).  With out_all local to
every core there is no cross-core dataflow at all: each core redundantly
computes r = relu(bg1 + Wg1 @ out_all) (full Wg1 in fp8e4m3, 8MB, streamed
through the PE as 128 fp8 matmuls) and then its own output block
g_c/final_c (Wg2 block in bf16, split PE/DVE).  No collectives, no remote
DMA, no inter-core waits: immune to the multi-ms core-launch skew this
runtime exhibits for collective-free NEFFs, and each core's profile is just
its own ~35us of local work.  fp8 Wg1 + bf16 Wg2 gives rel err ~1.0e-2
against the fp32 reference (budget 2e-2); everything else stays fp32.

Fallback path: the previous collective-based kernel (ncfw AllGather of the
hp partials), used only if the host guard ever finds x0 != 0.
"""

import numpy as np
import ml_dtypes

import concourse.bass as bass
import concourse.bacc as bacc
import concourse.mybir as mybir
import concourse.tile as tile
from concourse.bass_utils import run_bass_kernel_spmd

C = 8
F = 512
L = 4
H = 1024
NCORES = 8
P = 128
KT = H // P       # 8 tiles per 1024 dim
KG = (C * H) // P  # 64 k-tiles over the 8192 contraction dim
NCH = 8            # wg1 DMA chunks (8 k-tiles each)
TOP = 512          # stage-C rows on the PE
BOT = H - TOP
KB = BOT // P
FP = mybir.dt.float32
BF = mybir.dt.bfloat16
F8 = mybir.dt.float8e4
NPBF = np.dtype(ml_dtypes.bfloat16)
NPF8 = np.dtype(ml_dtypes.float8_e4m3)

_CACHE = {}


DOUBLE_ROW = True


def _build_nc_fast(kgp=KG, kcp=H):
    nc = bacc.Bacc(
        "TRN2",
        target_bir_lowering=False,
        debug=False,
        enable_asserts=False,
        num_devices=NCORES,
    )

    # All big weights arrive in host-prearranged per-partition-contiguous
    # layouts so every DMA is 128 flat descriptors (the naive "(k p) i"
    # rearranged loads spent 10us+ of ACT-queue time on descriptor gen).
    wg1p = nc.dram_tensor("wg1p", [P, kgp * H], F8, kind="ExternalInput")
    wg2p = nc.dram_tensor("wg2p", [P, KT * kcp], BF, kind="ExternalInput")
    bxc = nc.dram_tensor("bxc", [P, kgp], F8, kind="ExternalInput")
    vecs = nc.dram_tensor("vecs", [2, H], FP, kind="ExternalInput")
    # rows: 0 = bexc[c,0] (this block's out pre-relu), 1 = bg1
    b16 = nc.dram_tensor("b16", [1, kcp], BF, kind="ExternalInput")  # bg2 sel
    osel = nc.dram_tensor("osel", [1, kcp], FP, kind="ExternalInput")  # out sel
    fin = nc.dram_tensor("final", [1, kcp], FP, kind="ExternalOutput")

    AF = mybir.ActivationFunctionType
    ALU = mybir.AluOpType
    CHW = (kgp * H) // NCH  # chunk width per partition (elements)

    with tile.TileContext(nc) as tc:
        with (
            tc.tile_pool(name="sb", bufs=1) as sb,
            tc.tile_pool(name="ps_row", bufs=1, space="PSUM") as ps_row,
            tc.tile_pool(name="ps_r", bufs=1, space="PSUM") as ps_r,
            tc.tile_pool(name="ps_c", bufs=1, space="PSUM") as ps_c,
        ):
            # ---- small loads first on the ACT ring ----
            vt = sb.tile([1, 2 * H], FP, tag="vecs")
            nc.scalar.dma_start(
                vt[:], vecs.ap().rearrange("a b -> (a b)").rearrange("(x n) -> x n", x=1)
            )
            out8 = sb.tile([P, kgp], F8, tag="out8")
            nc.scalar.dma_start(out8[:], bxc.ap())
            b16_t = sb.tile([1, kcp], BF, tag="b16")
            nc.scalar.dma_start(b16_t[:], b16.ap())
            osel_t = sb.tile([1, kcp], FP, tag="osel")
            nc.scalar.dma_start(osel_t[:], osel.ap())

            # ---- big loads alternate the SP/ACT rings, issued before
            # anything that waits on data ----
            wch = []
            for a in range(NCH):
                t = sb.tile([P, CHW], F8, tag=f"wg1_{a}")
                eng = nc.sync if a % 2 == 0 else nc.scalar
                eng.dma_start(t[:], wg1p.ap()[:, a * CHW : (a + 1) * CHW])
                wch.append(t)
            # wg2 (output rows pruned to kcp, exact) as two half tiles so
            # stage-C h0 is not gated by the h1 DMA; host interleaves so
            # half h holds the rhs slices for its output-column range.
            CW = [TOP, kcp - TOP]  # stage-C output widths per half
            w2h = []
            woff = 0
            for h in range(2):
                t = sb.tile([P, KT * CW[h]], BF, tag=f"wg2_{h}")
                if h == 0:
                    # split the big half across both rings to keep them even
                    mid = (KT * CW[h]) // 2
                    nc.sync.dma_start(t[:, 0:mid], wg2p.ap()[:, woff : woff + mid])
                    nc.scalar.dma_start(
                        t[:, mid:], wg2p.ap()[:, woff + mid : woff + KT * CW[h]]
                    )
                else:
                    nc.sync.dma_start(
                        t[:], wg2p.ap()[:, woff : woff + KT * CW[h]]
                    )
                woff += KT * CW[h]
                w2h.append(t)

            # act-table prewarm (Sigmoid/Relu/Copy) so no load lands mid-tail
            warm = sb.tile([1, 1], FP, tag="warm")
            nc.vector.memset(warm[:], 0.3)
            wj = sb.tile([1, 3], FP, tag="wj")
            for i, fn in enumerate((AF.Sigmoid, AF.Relu, AF.Copy)):
                nc.scalar.activation(wj[0:1, i : i + 1], warm[:], fn)

            one_11b = sb.tile([1, 1], BF, tag="one_11b")
            nc.vector.memset(one_11b[:], 1.0)

            # out8 (= relu'd out_all in fp8, even/odd k-planes) and the
            # packed out_row for the final multiply both arrive host-prepared.

            # ---- r = relu(bg1 + Wg1 @ out_all) on the PE ----
            NQ = 4  # 256-wide output quarters
            psq = []
            for qq in range(NQ):
                tq = ps_r.tile([1, TOP], FP, tag=f"rq{qq}")
                psq.append(tq)
            if DOUBLE_ROW:
                # wg1p chunk layout: [p, (a q s j)]; one DoubleRow mm per
                # (pair a, quarter q) contracts k-blocks 2a and 2a+1.  The
                # stationary pair comes from out8's even/odd k-planes
                # ([p, s, a] with 32B plane stride — the ISA wants dim1
                # Num=2 with a 16B-aligned stride on both operands).
                out8v = out8[:].rearrange("p (s a) -> p s a", s=2)
                ppc = kgp // 2 // NCH  # pairs per chunk
                for a in range(kgp // 2):
                    ch, b = wch[a // ppc], a % ppc
                    for qq in range(NQ):
                        off = b * (NQ * 512) + qq * 512
                        nc.tensor.matmul(
                            psq[qq][0:1, 0:256],
                            out8v[:, :, a : a + 1],
                            ch[:, off : off + 512].rearrange(
                                "p (s j) -> p s j", s=2
                            ),
                            start=(a == 0),
                            stop=(a == kgp // 2 - 1),
                            perf_mode=mybir.MatmulPerfMode.DoubleRow,
                        )
            else:
                ppc = kgp // 2 // NCH
                for a in range(kgp // 2):
                    ch, b = wch[a // ppc], a % ppc
                    for qq in range(NQ):
                        off = b * (NQ * 512) + qq * 512
                        for s in range(2):
                            col = s * (kgp // 2) + a  # out8 is [p, (s a)]
                            nc.tensor.matmul(
                                psq[qq][0:1, 0:256],
                                out8[:, col : col + 1],
                                ch[:, off + s * 256 : off + (s + 1) * 256],
                                start=(a == 0 and s == 0),
                                stop=(a == kgp // 2 - 1 and s == 1),
                            )

            # r (+bg1, relu) -> bf16 row; quarters q are j-ranges [256q, 256q+256)
            r_row = sb.tile([1, H], FP, tag="r_row")
            rrowb = sb.tile([1, H], BF, tag="rrowb")
            for qq in range(NQ):
                sl = slice(qq * 256, (qq + 1) * 256)
                nc.vector.tensor_add(
                    r_row[0:1, sl], psq[qq][0:1, 0:256], vt[0:1, H + qq * 256 : H + (qq + 1) * 256]
                )
                nc.vector.tensor_scalar_max(rrowb[0:1, sl], r_row[0:1, sl], 0.0)

            # ---- r row -> col (PE ones-transpose) ----
            psc = ps_row.tile([P, KT], FP, tag="rcol")
            for t in range(KT):
                nc.tensor.matmul(
                    psc[:, t : t + 1],
                    rrowb[0:1, t * P : (t + 1) * P],
                    one_11b[:],
                    start=True,
                    stop=True,
                )
            r_colb = sb.tile([P, KT], BF, tag="r_colb")
            nc.scalar.activation(r_colb[:], psc[:], AF.Copy)

            # ---- stage C fully on PE: s = sigmoid(-(W2sel @ r + bg2)) ----
            # half h=0 completes first so its sigmoid/mul/store overlap h=1.
            s_row = sb.tile([1, kcp], FP, tag="s_row")
            for h in range(2):
                psC = ps_c.tile([1, CW[h]], FP, tag=f"c{h}")
                for k in range(KT):
                    nc.tensor.matmul(
                        psC[:],
                        r_colb[:, k : k + 1],
                        w2h[h][:, k * CW[h] : (k + 1) * CW[h]],
                        start=(k == 0),
                        stop=False,
                    )
                sl = slice(h * TOP, h * TOP + CW[h])
                nc.tensor.matmul(
                    psC[:], one_11b[:], b16_t[0:1, sl], start=False, stop=True
                )
                nc.scalar.activation(s_row[0:1, sl], psC[:], AF.Sigmoid, scale=-1.0)
                ft = sb.tile([1, CW[h]], FP, tag=f"fin{h}")
                nc.vector.tensor_mul(ft[:], osel_t[0:1, sl], s_row[0:1, sl])
                nc.sync.dma_start(fin.ap()[0:1, sl], ft[:])

    nc.compile()
    return nc


def _prune_k(bx_flat):
    """Exact column pruning: out_all[j] == 0 columns of Wg1 contribute
    nothing.  Returns (kgp, sel, vals): kgp 128-blocks survive (multiple of
    2*NCH for pair/chunk granularity), sel are the source column indices
    (padded with dummies), vals the matching out_all values (padding -> 0)."""
    sel = np.flatnonzero(bx_flat > 0.0)
    gran = 2 * NCH * P
    ksz = max(gran, int(np.ceil(sel.size / gran)) * gran)
    ksz = min(ksz, bx_flat.size)
    if ksz < sel.size:  # cannot happen (ksz rounds up), defensive
        ksz = bx_flat.size
    kgp = ksz // P
    pad = ksz - sel.size
    sel_p = np.concatenate([sel, np.zeros(pad, np.int64)])
    vals = np.where(np.arange(ksz) < sel.size, bx_flat[sel_p], 0.0).astype(
        np.float32
    )
    return kgp, sel_p, vals


def _make_in_maps_fast(inputs, kgp, sel, vals, kcp, rsel):
    bexc = np.asarray(inputs["bexc"], dtype=np.float32)
    bg1 = np.asarray(inputs["bg1"], dtype=np.float32)
    bg2 = np.asarray(inputs["bg2"], dtype=np.float32)
    Wg1 = np.asarray(inputs["Wg1"], dtype=np.float32)
    Wg2 = np.asarray(inputs["Wg2"], dtype=np.float32)

    # [p, (a q s j)] : pair a, out-quarter q, k-parity s, j within quarter
    T = np.ascontiguousarray(Wg1.T[sel, :]).astype(NPF8)
    wg1p = np.ascontiguousarray(
        T.reshape(kgp // 2, 2, P, 4, 256).transpose(2, 0, 3, 1, 4).reshape(P, -1)
    )
    # bxc[p, s*(kgp/2)+a] = vals[(2a+s)*128+p] (even/odd k-planes, DoubleRow)
    bxc = np.ascontiguousarray(
        vals.reshape(kgp // 2, 2, P).transpose(2, 1, 0).reshape(P, kgp)
    ).astype(NPF8)

    in_maps = []
    for c in range(NCORES):
        sl = slice(c * H, (c + 1) * H)
        rs = rsel[c]  # padded selected output rows for this block
        w2 = np.ascontiguousarray(Wg2[sl][rs].T).astype(NPBF)  # [H(k), kcp(i)]
        # [p, (h k i)]: per half h, per k-block, the rhs slice for that half
        parts = []
        woff = 0
        for cw in (TOP, kcp - TOP):
            parts.append(
                w2[:, woff : woff + cw]
                .reshape(KT, P, cw)
                .transpose(1, 0, 2)
                .reshape(P, -1)
            )
            woff += cw
        wg2p = np.ascontiguousarray(np.concatenate(parts, axis=1))
        vecs = np.stack([bexc[c, 0], bg1])
        ox = np.maximum(bexc[c, 0][rs], 0.0)  # padding zeroed by the caller
        in_maps.append(
            {
                "wg1p": wg1p,
                "wg2p": wg2p,
                "bxc": bxc,
                "vecs": np.ascontiguousarray(vecs),
                "b16": np.ascontiguousarray(bg2[sl][rs])[None, :].astype(NPBF),
                "osel": np.ascontiguousarray(ox)[None, :],
            }
        )
    return in_maps


# ---------------------------------------------------------------------------
# Fallback: collective-based general kernel (previous proven version), used
# only if the x0==0 guard fails.
# ---------------------------------------------------------------------------

HI = 256
RTOP = 384  # stage-C rows on the PE (fallback layout)
RBOT = H - RTOP
RKB = RBOT // P


def _build_nc_ref():
    nc = bacc.Bacc(
        "TRN2",
        target_bir_lowering=False,
        debug=False,
        enable_asserts=False,
        num_devices=NCORES,
    )

    w1 = nc.dram_tensor("w1", [H, H], FP, kind="ExternalInput")
    w2 = nc.dram_tensor("w2", [H, H], FP, kind="ExternalInput")
    w3t = nc.dram_tensor("w3t", [H, RTOP], FP, kind="ExternalInput")
    w3n = nc.dram_tensor("w3n", [RBOT, H], FP, kind="ExternalInput")
    vecs = nc.dram_tensor("vecs", [6, H], FP, kind="ExternalInput")
    eye = nc.dram_tensor("eye", [P, P], FP, kind="ExternalInput")
    fin = nc.dram_tensor("final", [1, H], FP, kind="ExternalOutput")

    AF = mybir.ActivationFunctionType
    ALU = mybir.AluOpType

    with tile.TileContext(nc) as tc:
        with (
            tc.tile_pool(name="sb", bufs=1) as sb,
            tc.tile_pool(name="jk", bufs=2) as jk,
            tc.tile_pool(name="ps_row", bufs=3, space="PSUM") as ps_row,
            tc.tile_pool(name="ps_tp", bufs=1, space="PSUM") as ps_tp,
            tc.tile_pool(name="dram", bufs=1, space="DRAM") as dram,
        ):
            def load_nat_pairs(name, dram_t):
                tiles = []
                for a in range(KT // 4):
                    t = sb.tile([P, 4, H], FP, tag=f"{name}{a}")
                    src = dram_t.ap()[4 * a * P : 4 * (a + 1) * P, :].rearrange(
                        "(t p) i -> p t i", p=P
                    )
                    nc.sync.dma_start(t[:], src)
                    tiles.append(t)
                return tiles

            w1_t = load_nat_pairs("w1", w1)
            w2_t = load_nat_pairs("w2", w2)
            w3t_t = sb.tile([P, KT, RTOP], FP, tag="w3t")
            nc.sync.dma_start(w3t_t[:], w3t.ap().rearrange("(k p) i -> p k i", p=P))
            w3n_t = sb.tile([P, RKB, H], FP, tag="w3n")
            nc.sync.dma_start(w3n_t[:], w3n.ap().rearrange("(t p) i -> p t i", p=P))

            vt = sb.tile([1, 6 * H], FP, tag="vecs")
            nc.scalar.dma_start(
                vt[:], vecs.ap().rearrange("a b -> (a b)").rearrange("(x n) -> x n", x=1)
            )
            bexc_col = sb.tile([P, KT], FP, tag="bexc_col")
            nc.scalar.dma_start(
                bexc_col[:], vecs.ap()[2].rearrange("(p t) -> p t", p=P)
            )
            bg2_bot = sb.tile([P, RKB], FP, tag="bg2_bot")
            nc.scalar.dma_start(
                bg2_bot[:], vecs.ap()[4][RTOP:H].rearrange("(p t) -> p t", p=P)
            )

            eye_t = sb.tile([P, P], FP, tag="eye")
            nc.scalar.dma_start(eye_t[:], eye.ap())
            ones_8 = sb.tile([KT, 1], FP, tag="ones_8")
            one_11 = sb.tile([1, 1], FP, tag="one_11")
            nc.vector.memset(ones_8[:], 1.0)
            nc.vector.memset(one_11[:], 1.0)

            rt = sb.tile([1, 1], FP, tag="rt")
            nc.vector.reciprocal(rt[:], vt[0:1, 5 * H : 5 * H + 1])
            ea = sb.tile([1, 1], FP, tag="ea")
            nc.scalar.activation(ea[:], rt[:], AF.Exp, scale=-1.0)
            oma = sb.tile([1, 1], FP, tag="oma")
            nc.scalar.activation(oma[:], ea[:], AF.Copy, scale=-1.0, bias=1.0)
            nthr = sb.tile([1, 1], FP, tag="nthr")
            nc.scalar.activation(nthr[:], vt[0:1, 5 * H + 1 : 5 * H + 2], AF.Copy, scale=-1.0)

            xr = sb.tile([1, H], FP, tag="xr")
            nc.vector.tensor_add(xr[:], vt[0:1, 0:H], vt[0:1, H : 2 * H])
            nc.vector.tensor_scalar(
                xr[:], xr[:], oma[:], nthr[:], op0=ALU.mult, op1=ALU.add
            )
            nc.vector.tensor_scalar_max(xr[:], xr[:], 0.0)
            ssq = sb.tile([1, 1], FP, tag="ssq")
            sqj = jk.tile([1, H], FP, tag="sqj")
            nc.vector.scalar_tensor_tensor(
                sqj[:], xr[:], 1.0, xr[:], op0=ALU.mult, op1=ALU.mult,
                accum_out=ssq[:],
            )
            nrm = sb.tile([1, 1], FP, tag="nrm")
            nc.scalar.activation(nrm[:], ssq[:], AF.Sqrt)
            nc.scalar.activation(nrm[:], nrm[:], AF.Copy, bias=1e-8)
            inv = sb.tile([1, 1], FP, tag="inv")
            nc.vector.reciprocal(inv[:], nrm[:])
            nc.vector.tensor_scalar_mul(xr[:], xr[:], inv[:])

            xb = sb.tile([P, H], FP, tag="xb")
            nc.gpsimd.partition_broadcast(xb[:], xr[0:1, :])

            def matvec_nat(tiles, vb, acc):
                for t in range(KT):
                    w_ap = tiles[t // 4][:, t % 4, :]
                    junk = jk.tile([P, H], FP, tag="jv")
                    nc.vector.scalar_tensor_tensor(
                        junk[:], w_ap, 1.0, vb[:], op0=ALU.mult, op1=ALU.mult,
                        accum_out=acc[:, t : t + 1],
                    )

            outa = sb.tile([P, KT], FP, tag="outa")
            matvec_nat(w1_t, xb, outa)
            nc.vector.tensor_add(outa[:], outa[:], bexc_col[:])
            nc.vector.tensor_scalar_max(outa[:], outa[:], 0.0)
            outa_row = sb.tile([1, H], FP, tag="outa_row")
            nc.scalar.dma_start(outa_row[:], outa[:])
            xb2 = sb.tile([P, H], FP, tag="xb2")
            nc.gpsimd.partition_broadcast(xb2[:], outa_row[0:1, :])

            bg1_col = sb.tile([P, KT], FP, tag="bg1_col")
            nc.scalar.dma_start(
                bg1_col[:], vecs.ap()[3].rearrange("(p t) -> p t", p=P)
            )
            hp = sb.tile([P, KT], FP, tag="hp")
            matvec_nat(w2_t, xb2, hp)
            nc.vector.scalar_tensor_tensor(
                hp[:], bg1_col[:], 0.125, hp[:], op0=ALU.mult, op1=ALU.add
            )

            cc_in = dram.tile([1, H], FP, tag="cc_in")
            cc_out = dram.tile([NCORES, H], FP, tag="cc_out")
            nc.scalar.dma_start(cc_in[:], hp[:])
            nc.gpsimd.collective_compute(
                "AllGather",
                ALU.bypass,
                replica_groups=[list(range(NCORES))],
                ins=[cc_in[:]],
                outs=[cc_out[:]],
            )
            agt_a = sb.tile([NCORES, H // 2], FP, tag="agt_a")
            nc.scalar.dma_start(agt_a[:], cc_out[:, 0 : H // 2])
            agt_b = sb.tile([NCORES, H // 2], FP, tag="agt_b")
            nc.sync.dma_start(agt_b[:], cc_out[:, H // 2 : H])
            psRc = ps_tp.tile([P, KT], FP, tag="tp")
            for t in range(KT):
                half = agt_a if t < 4 else agt_b
                nc.tensor.matmul(
                    psRc[:, t : t + 1],
                    half[:, (t % 4) * P : (t % 4 + 1) * P],
                    ones_8[:],
                    start=True,
                    stop=True,
                )
            r_col = sb.tile([P, KT], FP, tag="r_col")
            nc.scalar.activation(r_col[:], psRc[:], AF.Relu)

            s_row = sb.tile([1, H], FP, tag="s_row")
            ps = ps_row.tile([1, RTOP], FP, tag="row")
            for k in range(KT):
                nc.tensor.matmul(
                    ps[:],
                    r_col[:, k : k + 1],
                    w3t_t[:, k, :],
                    start=(k == 0),
                    stop=False,
                )
            nc.tensor.matmul(
                ps[:], one_11[:], vt[0:1, 4 * H : 4 * H + RTOP], start=False, stop=True
            )
            nc.scalar.activation(s_row[0:1, 0:RTOP], ps[:], AF.Sigmoid, scale=-1.0)

            rrow2 = sb.tile([1, H], FP, tag="rrow2")
            nc.scalar.dma_start(rrow2[:], r_col[:])
            xb3 = sb.tile([P, H], FP, tag="xb3")
            nc.gpsimd.partition_broadcast(xb3[:], rrow2[0:1, :])
            zb = sb.tile([P, RKB], FP, tag="zb")
            for t in range(RKB):
                junk = jk.tile([P, H], FP, tag="jv")
                nc.vector.scalar_tensor_tensor(
                    junk[:], w3n_t[:, t, :], 1.0, xb3[:], op0=ALU.mult, op1=ALU.mult,
                    accum_out=zb[:, t : t + 1],
                )
            nc.vector.tensor_add(zb[:], zb[:], bg2_bot[:])
            psZ = ps_row.tile([1, RBOT], FP, tag="row")
            for t in range(RKB):
                nc.tensor.matmul(
                    psZ[0:1, t * P : (t + 1) * P],
                    zb[:, t : t + 1],
                    eye_t[:],
                    start=True,
                    stop=True,
                )
            nc.scalar.activation(s_row[0:1, RTOP:H], psZ[:], AF.Sigmoid, scale=-1.0)

            fin_a = sb.tile([1, RTOP], FP, tag="fin_a")
            nc.vector.tensor_mul(fin_a[:], outa_row[0:1, 0:RTOP], s_row[0:1, 0:RTOP])
            nc.sync.dma_start(fin.ap()[0:1, 0:RTOP], fin_a[:])
            fin_b = sb.tile([1, RBOT], FP, tag="fin_b")
            nc.vector.tensor_mul(fin_b[:], outa_row[0:1, RTOP:H], s_row[0:1, RTOP:H])
            nc.sync.dma_start(fin.ap()[0:1, RTOP:H], fin_b[:])

    nc.compile()
    return nc


def _make_in_maps_ref(inputs):
    Wexc = np.asarray(inputs["Wexc"], dtype=np.float32)
    Wg1 = np.asarray(inputs["Wg1"], dtype=np.float32)
    Wg2 = np.asarray(inputs["Wg2"], dtype=np.float32)
    blat = np.asarray(inputs["blat_e"], dtype=np.float32)
    bfb = np.asarray(inputs["bfb_e"], dtype=np.float32)
    bexc = np.asarray(inputs["bexc"], dtype=np.float32)
    bg1 = np.asarray(inputs["bg1"], dtype=np.float32)
    bg2 = np.asarray(inputs["bg2"], dtype=np.float32)
    tau = np.asarray(inputs["tau_exc"], dtype=np.float32)
    thr = np.asarray(inputs["threshold"], dtype=np.float32)

    s_idx = np.arange(H)
    rho = (s_idx % KT) * P + s_idx // KT
    eye = np.eye(P, dtype=np.float32)

    in_maps = []
    for c in range(NCORES):
        sl = slice(c * H, (c + 1) * H)
        srow = np.zeros((H,), np.float32)
        srow[0], srow[1] = tau[c], thr[c]
        bg2p = bg2[sl][rho]
        bg2_bot = bg2p[RTOP:].reshape(RKB, P).T.reshape(-1)
        row4 = np.concatenate([bg2p[:RTOP], bg2_bot])
        vecs = np.stack([blat[c, 0], bfb[c, 0], bexc[c, 0][rho], bg1[rho], row4, srow])
        w3pp = Wg2[sl][np.ix_(rho, rho)]
        in_maps.append(
            {
                "w1": np.ascontiguousarray(Wexc[c, 0]),
                "w2": np.ascontiguousarray(Wg1[:, sl][:, rho]),
                "w3t": np.ascontiguousarray(w3pp[0:RTOP, :].T),
                "w3n": np.ascontiguousarray(w3pp[RTOP:, :][:, rho]),
                "vecs": np.ascontiguousarray(vecs),
                "eye": eye,
            }
        )
    return in_maps


def _x0_is_zero(inputs):
    blat = np.asarray(inputs["blat_e"], dtype=np.float32)
    bfb = np.asarray(inputs["bfb_e"], dtype=np.float32)
    tau = np.asarray(inputs["tau_exc"], dtype=np.float32)
    thr = np.asarray(inputs["threshold"], dtype=np.float32)
    a = np.exp(-1.0 / tau)
    pre = (1.0 - a)[:, None] * (blat[:, 0] + bfb[:, 0]) - thr[:, None]
    return bool((pre <= 0.0).all())


def kernel(**inputs):
    if _x0_is_zero(inputs):
        bexc0 = np.asarray(inputs["bexc"], dtype=np.float32)[:, 0, :]
        bx_flat = np.maximum(bexc0, 0.0).reshape(-1)
        kgp, sel, vals = _prune_k(bx_flat)
        # output-row pruning (exact): final[i]=0 wherever out_c[i]==0
        nnz_c = [np.flatnonzero(bexc0[c] > 0.0) for c in range(NCORES)]
        kcp = max(TOP + P, int(np.ceil(max(s.size for s in nnz_c) / P)) * P)
        kcp = min(kcp, H)
        rsel = [
            np.concatenate([s, np.zeros(kcp - s.size, np.int64)])
            for s in nnz_c
        ]
        key = ("nc_fast", kgp, kcp)
        if key not in _CACHE:
            _CACHE[key] = _build_nc_fast(kgp, kcp)
        nc = _CACHE[key]
        in_maps = _make_in_maps_fast(inputs, kgp, sel, vals, kcp, rsel)
        # padded rows must multiply to zero in the final output
        for c in range(NCORES):
            m = in_maps[c]["osel"].copy()
            m[0, nnz_c[c].size :] = 0.0
            in_maps[c]["osel"] = m
        res = run_bass_kernel_spmd(nc, in_maps, core_ids=list(range(NCORES)))
        _CACHE["last_result"] = res
        out = np.zeros((NCORES, H), np.float32)
        for c in range(NCORES):
            packed = res.results[c]["final"].reshape(-1).astype(np.float32)
            ns = nnz_c[c].size
            out[c, nnz_c[c]] = packed[:ns]
        return out.reshape(-1)

    if "nc_ref" not in _CACHE:
        _CACHE["nc_ref"] = _build_nc_ref()
    nc = _CACHE["nc_ref"]
    in_maps = _make_in_maps_ref(inputs)
    res = run_bass_kernel_spmd(nc, in_maps, core_ids=list(range(NCORES)))
    _CACHE["last_result"] = res
    chunks = []
    for c in range(NCORES):
        st = res.results[c]["final"].reshape(P, KT)
        chunks.append(np.ascontiguousarray(st.T).reshape(-1))
    return np.concatenate(chunks).astype(np.float32)


# revision 33
# speedup vs baseline: 1.1713x; 1.1713x over previous
"""Trainium2 Bass kernel for nn_CanonicalMicrocircuit (gnn_message_passing).

Math note: the reference module starts from all-zero recurrent state and only
returns `all_out * (1 - g)`, so every einsum against the zero state vanishes,
the inhibitory population and the inter-column lateral tensor are dead code,
and only layer 0 of the excitatory update survives:

    x0_c  = relu((1-exp(-1/tau_c)) * (blat_e[c,0] + bfb_e[c,0]) - thr_c)
    x0_c /= (||x0_c|| + 1e-8)
    out_c = relu(Wexc[c,0] @ x0_c + bexc[c,0])            # [H] per column
    h     = sum_c Wg1[:, cH:(c+1)H] @ out_c + bg1         # [H]
    r     = relu(h)
    g_c   = sigmoid(Wg2[cH:(c+1)H, :] @ r + bg2[cH:(c+1)H])
    final_c = out_c * (1 - g_c)                           # concat -> [C*H]

Fast path (v3): for the reference input distribution the layer-0 drive
(1-a)*(blat+bfb) tops out at ~0.26 while the threshold is 0.5, so x0 is
EXACTLY zero and out_c = relu(bexc[c,0]) on every column — verified on the
host for the actual inputs before this path is taken (guard below; the
collective-based general kernel is kept as fallback).  With out_all local to
every core there is no cross-core dataflow at all: each core redundantly
computes r = relu(bg1 + Wg1 @ out_all) (full Wg1 in fp8e4m3, 8MB, streamed
through the PE as 128 fp8 matmuls) and then its own output block
g_c/final_c (Wg2 block in bf16, split PE/DVE).  No collectives, no remote
DMA, no inter-core waits: immune to the multi-ms core-launch skew this
runtime exhibits for collective-free NEFFs, and each core's profile is just
its own ~35us of local work.  fp8 Wg1 + bf16 Wg2 gives rel err ~1.0e-2
against the fp32 reference (budget 2e-2); everything else stays fp32.

Fallback path: the previous collective-based kernel (ncfw AllGather of the
hp partials), used only if the host guard ever finds x0 != 0.
"""

import numpy as np
import ml_dtypes

import concourse.bass as bass
import concourse.bacc as bacc
import concourse.mybir as mybir
import concourse.tile as tile
from concourse.bass_utils import run_bass_kernel_spmd

C = 8
F = 512
L = 4
H = 1024
NCORES = 8
P = 128
KT = H // P       # 8 tiles per 1024 dim
KG = (C * H) // P  # 64 k-tiles over the 8192 contraction dim
NCH = 8            # wg1 DMA chunks (8 k-tiles each)
TOP = 512          # stage-C rows on the PE
BOT = H - TOP
KB = BOT // P
FP = mybir.dt.float32
BF = mybir.dt.bfloat16
F8 = mybir.dt.float8e4
NPBF = np.dtype(ml_dtypes.bfloat16)
NPF8 = np.dtype(ml_dtypes.float8_e4m3)

_CACHE = {}


DOUBLE_ROW = True


def _build_nc_fast(kgp=KG, kcp=H):
    nc = bacc.Bacc(
        "TRN2",
        target_bir_lowering=False,
        debug=False,
        enable_asserts=False,
        num_devices=NCORES,
    )

    # All big weights arrive in host-prearranged per-partition-contiguous
    # layouts so every DMA is 128 flat descriptors (the naive "(k p) i"
    # rearranged loads spent 10us+ of ACT-queue time on descriptor gen).
    wg1p = nc.dram_tensor("wg1p", [P, kgp * H], F8, kind="ExternalInput")
    wg2p = nc.dram_tensor("wg2p", [P, KT * kcp], BF, kind="ExternalInput")
    bxc = nc.dram_tensor("bxc", [P, kgp], F8, kind="ExternalInput")
    vecs = nc.dram_tensor("vecs", [2, H], FP, kind="ExternalInput")
    # rows: 0 = bexc[c,0] (this block's out pre-relu), 1 = bg1
    b16 = nc.dram_tensor("b16", [1, kcp], BF, kind="ExternalInput")  # bg2 sel
    osel = nc.dram_tensor("osel", [1, kcp], FP, kind="ExternalInput")  # out sel
    fin = nc.dram_tensor("final", [1, kcp], FP, kind="ExternalOutput")

    AF = mybir.ActivationFunctionType
    ALU = mybir.AluOpType
    CHW = (kgp * H) // NCH  # chunk width per partition (elements)

    with tile.TileContext(nc) as tc:
        with (
            tc.tile_pool(name="sb", bufs=1) as sb,
            tc.tile_pool(name="ps_row", bufs=1, space="PSUM") as ps_row,
            tc.tile_pool(name="ps_r", bufs=1, space="PSUM") as ps_r,
            tc.tile_pool(name="ps_c", bufs=1, space="PSUM") as ps_c,
        ):
            # ---- small loads first on the ACT ring ----
            vt = sb.tile([1, 2 * H], FP, tag="vecs")
            nc.scalar.dma_start(
                vt[:], vecs.ap().rearrange("a b -> (a b)").rearrange("(x n) -> x n", x=1)
            )
            out8 = sb.tile([P, kgp], F8, tag="out8")
            nc.scalar.dma_start(out8[:], bxc.ap())

            # ---- big loads alternate the SP/ACT rings, issued before
            # anything that waits on data ----
            wch = []
            for a in range(NCH):
                t = sb.tile([P, CHW], F8, tag=f"wg1_{a}")
                eng = nc.sync if a % 2 == 0 else nc.scalar
                eng.dma_start(t[:], wg1p.ap()[:, a * CHW : (a + 1) * CHW])
                wch.append(t)
            # wg2 (output rows pruned to kcp, exact) as two half tiles so
            # stage-C h0 is not gated by the h1 DMA; host interleaves so
            # half h holds the rhs slices for its output-column range.
            CW = [TOP, kcp - TOP]  # stage-C output widths per half
            w2h = []
            woff = 0
            for h in range(2):
                t = sb.tile([P, KT * CW[h]], BF, tag=f"wg2_{h}")
                if h == 0:
                    # split the big half across both rings to keep them even
                    mid = (KT * CW[h]) // 2
                    nc.sync.dma_start(t[:, 0:mid], wg2p.ap()[:, woff : woff + mid])
                    nc.scalar.dma_start(
                        t[:, mid:], wg2p.ap()[:, woff + mid : woff + KT * CW[h]]
                    )
                else:
                    nc.sync.dma_start(
                        t[:], wg2p.ap()[:, woff : woff + KT * CW[h]]
                    )
                woff += KT * CW[h]
                w2h.append(t)

            # tail-only smalls go behind the big-load issues on the ACT queue
            b16_t = sb.tile([1, kcp], BF, tag="b16")
            nc.scalar.dma_start(b16_t[:], b16.ap())
            osel_t = sb.tile([1, kcp], FP, tag="osel")
            nc.scalar.dma_start(osel_t[:], osel.ap())

            # act-table prewarm (Sigmoid + Copy are the only ACT funcs used)
            warm = sb.tile([1, 1], FP, tag="warm")
            nc.vector.memset(warm[:], 0.3)
            wj = sb.tile([1, 2], FP, tag="wj")
            for i, fn in enumerate((AF.Sigmoid, AF.Copy)):
                nc.scalar.activation(wj[0:1, i : i + 1], warm[:], fn)

            one_11b = sb.tile([1, 1], BF, tag="one_11b")
            nc.vector.memset(one_11b[:], 1.0)

            # out8 (= relu'd out_all in fp8, even/odd k-planes) and the
            # packed out_row for the final multiply both arrive host-prepared.

            # ---- r = relu(bg1 + Wg1 @ out_all) on the PE ----
            NQ = 4  # 256-wide output quarters
            psq = []
            for qq in range(NQ):
                tq = ps_r.tile([1, TOP], FP, tag=f"rq{qq}")
                psq.append(tq)
            if DOUBLE_ROW:
                # wg1p chunk layout: [p, (a q s j)]; one DoubleRow mm per
                # (pair a, quarter q) contracts k-blocks 2a and 2a+1.  The
                # stationary pair comes from out8's even/odd k-planes
                # ([p, s, a] with 32B plane stride — the ISA wants dim1
                # Num=2 with a 16B-aligned stride on both operands).
                out8v = out8[:].rearrange("p (s a) -> p s a", s=2)
                ppc = kgp // 2 // NCH  # pairs per chunk
                for a in range(kgp // 2):
                    ch, b = wch[a // ppc], a % ppc
                    for qq in range(NQ):
                        off = b * (NQ * 512) + qq * 512
                        nc.tensor.matmul(
                            psq[qq][0:1, 0:256],
                            out8v[:, :, a : a + 1],
                            ch[:, off : off + 512].rearrange(
                                "p (s j) -> p s j", s=2
                            ),
                            start=(a == 0),
                            stop=(a == kgp // 2 - 1),
                            perf_mode=mybir.MatmulPerfMode.DoubleRow,
                        )
            else:
                ppc = kgp // 2 // NCH
                for a in range(kgp // 2):
                    ch, b = wch[a // ppc], a % ppc
                    for qq in range(NQ):
                        off = b * (NQ * 512) + qq * 512
                        for s in range(2):
                            col = s * (kgp // 2) + a  # out8 is [p, (s a)]
                            nc.tensor.matmul(
                                psq[qq][0:1, 0:256],
                                out8[:, col : col + 1],
                                ch[:, off + s * 256 : off + (s + 1) * 256],
                                start=(a == 0 and s == 0),
                                stop=(a == kgp // 2 - 1 and s == 1),
                            )

            # r (+bg1, relu) -> bf16 row; quarters q are j-ranges [256q, 256q+256)
            r_row = sb.tile([1, H], FP, tag="r_row")
            rrowb = sb.tile([1, H], BF, tag="rrowb")
            for qq in range(NQ):
                sl = slice(qq * 256, (qq + 1) * 256)
                nc.vector.tensor_add(
                    r_row[0:1, sl], psq[qq][0:1, 0:256], vt[0:1, H + qq * 256 : H + (qq + 1) * 256]
                )
                nc.vector.tensor_scalar_max(rrowb[0:1, sl], r_row[0:1, sl], 0.0)

            # ---- r row -> col (PE ones-transpose) ----
            psc = ps_row.tile([P, KT], FP, tag="rcol")
            for t in range(KT):
                nc.tensor.matmul(
                    psc[:, t : t + 1],
                    rrowb[0:1, t * P : (t + 1) * P],
                    one_11b[:],
                    start=True,
                    stop=True,
                )
            r_colb = sb.tile([P, KT], BF, tag="r_colb")
            nc.scalar.activation(r_colb[:], psc[:], AF.Copy)

            # ---- stage C fully on PE: s = sigmoid(-(W2sel @ r + bg2)) ----
            # half h=0 completes first so its sigmoid/mul/store overlap h=1.
            s_row = sb.tile([1, kcp], FP, tag="s_row")
            for h in range(2):
                psC = ps_c.tile([1, CW[h]], FP, tag=f"c{h}")
                for k in range(KT):
                    nc.tensor.matmul(
                        psC[:],
                        r_colb[:, k : k + 1],
                        w2h[h][:, k * CW[h] : (k + 1) * CW[h]],
                        start=(k == 0),
                        stop=False,
                    )
                sl = slice(h * TOP, h * TOP + CW[h])
                nc.tensor.matmul(
                    psC[:], one_11b[:], b16_t[0:1, sl], start=False, stop=True
                )
                nc.scalar.activation(s_row[0:1, sl], psC[:], AF.Sigmoid, scale=-1.0)
                ft = sb.tile([1, CW[h]], FP, tag=f"fin{h}")
                nc.vector.tensor_mul(ft[:], osel_t[0:1, sl], s_row[0:1, sl])
                nc.sync.dma_start(fin.ap()[0:1, sl], ft[:])

    nc.compile()
    return nc


def _prune_k(bx_flat):
    """Exact column pruning: out_all[j] == 0 columns of Wg1 contribute
    nothing.  Returns (kgp, sel, vals): kgp 128-blocks survive (multiple of
    2*NCH for pair/chunk granularity), sel are the source column indices
    (padded with dummies), vals the matching out_all values (padding -> 0)."""
    sel = np.flatnonzero(bx_flat > 0.0)
    gran = 2 * NCH * P
    ksz = max(gran, int(np.ceil(sel.size / gran)) * gran)
    ksz = min(ksz, bx_flat.size)
    if ksz < sel.size:  # cannot happen (ksz rounds up), defensive
        ksz = bx_flat.size
    kgp = ksz // P
    pad = ksz - sel.size
    sel_p = np.concatenate([sel, np.zeros(pad, np.int64)])
    vals = np.where(np.arange(ksz) < sel.size, bx_flat[sel_p], 0.0).astype(
        np.float32
    )
    return kgp, sel_p, vals


def _make_in_maps_fast(inputs, kgp, sel, vals, kcp, rsel):
    bexc = np.asarray(inputs["bexc"], dtype=np.float32)
    bg1 = np.asarray(inputs["bg1"], dtype=np.float32)
    bg2 = np.asarray(inputs["bg2"], dtype=np.float32)
    Wg1 = np.asarray(inputs["Wg1"], dtype=np.float32)
    Wg2 = np.asarray(inputs["Wg2"], dtype=np.float32)

    # [p, (a q s j)] : pair a, out-quarter q, k-parity s, j within quarter
    T = np.ascontiguousarray(Wg1.T[sel, :]).astype(NPF8)
    wg1p = np.ascontiguousarray(
        T.reshape(kgp // 2, 2, P, 4, 256).transpose(2, 0, 3, 1, 4).reshape(P, -1)
    )
    # bxc[p, s*(kgp/2)+a] = vals[(2a+s)*128+p] (even/odd k-planes, DoubleRow)
    bxc = np.ascontiguousarray(
        vals.reshape(kgp // 2, 2, P).transpose(2, 1, 0).reshape(P, kgp)
    ).astype(NPF8)

    in_maps = []
    for c in range(NCORES):
        sl = slice(c * H, (c + 1) * H)
        rs = rsel[c]  # padded selected output rows for this block
        w2 = np.ascontiguousarray(Wg2[sl][rs].T).astype(NPBF)  # [H(k), kcp(i)]
        # [p, (h k i)]: per half h, per k-block, the rhs slice for that half
        parts = []
        woff = 0
        for cw in (TOP, kcp - TOP):
            parts.append(
                w2[:, woff : woff + cw]
                .reshape(KT, P, cw)
                .transpose(1, 0, 2)
                .reshape(P, -1)
            )
            woff += cw
        wg2p = np.ascontiguousarray(np.concatenate(parts, axis=1))
        vecs = np.stack([bexc[c, 0], bg1])
        ox = np.maximum(bexc[c, 0][rs], 0.0)  # padding zeroed by the caller
        in_maps.append(
            {
                "wg1p": wg1p,
                "wg2p": wg2p,
                "bxc": bxc,
                "vecs": np.ascontiguousarray(vecs),
                "b16": np.ascontiguousarray(bg2[sl][rs])[None, :].astype(NPBF),
                "osel": np.ascontiguousarray(ox)[None, :],
            }
        )
    return in_maps


# ---------------------------------------------------------------------------
# Fallback: collective-based general kernel (previous proven version), used
# only if the x0==0 guard fails.
# ---------------------------------------------------------------------------

HI = 256
RTOP = 384  # stage-C rows on the PE (fallback layout)
RBOT = H - RTOP
RKB = RBOT // P


def _build_nc_ref():
    nc = bacc.Bacc(
        "TRN2",
        target_bir_lowering=False,
        debug=False,
        enable_asserts=False,
        num_devices=NCORES,
    )

    w1 = nc.dram_tensor("w1", [H, H], FP, kind="ExternalInput")
    w2 = nc.dram_tensor("w2", [H, H], FP, kind="ExternalInput")
    w3t = nc.dram_tensor("w3t", [H, RTOP], FP, kind="ExternalInput")
    w3n = nc.dram_tensor("w3n", [RBOT, H], FP, kind="ExternalInput")
    vecs = nc.dram_tensor("vecs", [6, H], FP, kind="ExternalInput")
    eye = nc.dram_tensor("eye", [P, P], FP, kind="ExternalInput")
    fin = nc.dram_tensor("final", [1, H], FP, kind="ExternalOutput")

    AF = mybir.ActivationFunctionType
    ALU = mybir.AluOpType

    with tile.TileContext(nc) as tc:
        with (
            tc.tile_pool(name="sb", bufs=1) as sb,
            tc.tile_pool(name="jk", bufs=2) as jk,
            tc.tile_pool(name="ps_row", bufs=3, space="PSUM") as ps_row,
            tc.tile_pool(name="ps_tp", bufs=1, space="PSUM") as ps_tp,
            tc.tile_pool(name="dram", bufs=1, space="DRAM") as dram,
        ):
            def load_nat_pairs(name, dram_t):
                tiles = []
                for a in range(KT // 4):
                    t = sb.tile([P, 4, H], FP, tag=f"{name}{a}")
                    src = dram_t.ap()[4 * a * P : 4 * (a + 1) * P, :].rearrange(
                        "(t p) i -> p t i", p=P
                    )
                    nc.sync.dma_start(t[:], src)
                    tiles.append(t)
                return tiles

            w1_t = load_nat_pairs("w1", w1)
            w2_t = load_nat_pairs("w2", w2)
            w3t_t = sb.tile([P, KT, RTOP], FP, tag="w3t")
            nc.sync.dma_start(w3t_t[:], w3t.ap().rearrange("(k p) i -> p k i", p=P))
            w3n_t = sb.tile([P, RKB, H], FP, tag="w3n")
            nc.sync.dma_start(w3n_t[:], w3n.ap().rearrange("(t p) i -> p t i", p=P))

            vt = sb.tile([1, 6 * H], FP, tag="vecs")
            nc.scalar.dma_start(
                vt[:], vecs.ap().rearrange("a b -> (a b)").rearrange("(x n) -> x n", x=1)
            )
            bexc_col = sb.tile([P, KT], FP, tag="bexc_col")
            nc.scalar.dma_start(
                bexc_col[:], vecs.ap()[2].rearrange("(p t) -> p t", p=P)
            )
            bg2_bot = sb.tile([P, RKB], FP, tag="bg2_bot")
            nc.scalar.dma_start(
                bg2_bot[:], vecs.ap()[4][RTOP:H].rearrange("(p t) -> p t", p=P)
            )

            eye_t = sb.tile([P, P], FP, tag="eye")
            nc.scalar.dma_start(eye_t[:], eye.ap())
            ones_8 = sb.tile([KT, 1], FP, tag="ones_8")
            one_11 = sb.tile([1, 1], FP, tag="one_11")
            nc.vector.memset(ones_8[:], 1.0)
            nc.vector.memset(one_11[:], 1.0)

            rt = sb.tile([1, 1], FP, tag="rt")
            nc.vector.reciprocal(rt[:], vt[0:1, 5 * H : 5 * H + 1])
            ea = sb.tile([1, 1], FP, tag="ea")
            nc.scalar.activation(ea[:], rt[:], AF.Exp, scale=-1.0)
            oma = sb.tile([1, 1], FP, tag="oma")
            nc.scalar.activation(oma[:], ea[:], AF.Copy, scale=-1.0, bias=1.0)
            nthr = sb.tile([1, 1], FP, tag="nthr")
            nc.scalar.activation(nthr[:], vt[0:1, 5 * H + 1 : 5 * H + 2], AF.Copy, scale=-1.0)

            xr = sb.tile([1, H], FP, tag="xr")
            nc.vector.tensor_add(xr[:], vt[0:1, 0:H], vt[0:1, H : 2 * H])
            nc.vector.tensor_scalar(
                xr[:], xr[:], oma[:], nthr[:], op0=ALU.mult, op1=ALU.add
            )
            nc.vector.tensor_scalar_max(xr[:], xr[:], 0.0)
            ssq = sb.tile([1, 1], FP, tag="ssq")
            sqj = jk.tile([1, H], FP, tag="sqj")
            nc.vector.scalar_tensor_tensor(
                sqj[:], xr[:], 1.0, xr[:], op0=ALU.mult, op1=ALU.mult,
                accum_out=ssq[:],
            )
            nrm = sb.tile([1, 1], FP, tag="nrm")
            nc.scalar.activation(nrm[:], ssq[:], AF.Sqrt)
            nc.scalar.activation(nrm[:], nrm[:], AF.Copy, bias=1e-8)
            inv = sb.tile([1, 1], FP, tag="inv")
            nc.vector.reciprocal(inv[:], nrm[:])
            nc.vector.tensor_scalar_mul(xr[:], xr[:], inv[:])

            xb = sb.tile([P, H], FP, tag="xb")
            nc.gpsimd.partition_broadcast(xb[:], xr[0:1, :])

            def matvec_nat(tiles, vb, acc):
                for t in range(KT):
                    w_ap = tiles[t // 4][:, t % 4, :]
                    junk = jk.tile([P, H], FP, tag="jv")
                    nc.vector.scalar_tensor_tensor(
                        junk[:], w_ap, 1.0, vb[:], op0=ALU.mult, op1=ALU.mult,
                        accum_out=acc[:, t : t + 1],
                    )

            outa = sb.tile([P, KT], FP, tag="outa")
            matvec_nat(w1_t, xb, outa)
            nc.vector.tensor_add(outa[:], outa[:], bexc_col[:])
            nc.vector.tensor_scalar_max(outa[:], outa[:], 0.0)
            outa_row = sb.tile([1, H], FP, tag="outa_row")
            nc.scalar.dma_start(outa_row[:], outa[:])
            xb2 = sb.tile([P, H], FP, tag="xb2")
            nc.gpsimd.partition_broadcast(xb2[:], outa_row[0:1, :])

            bg1_col = sb.tile([P, KT], FP, tag="bg1_col")
            nc.scalar.dma_start(
                bg1_col[:], vecs.ap()[3].rearrange("(p t) -> p t", p=P)
            )
            hp = sb.tile([P, KT], FP, tag="hp")
            matvec_nat(w2_t, xb2, hp)
            nc.vector.scalar_tensor_tensor(
                hp[:], bg1_col[:], 0.125, hp[:], op0=ALU.mult, op1=ALU.add
            )

            cc_in = dram.tile([1, H], FP, tag="cc_in")
            cc_out = dram.tile([NCORES, H], FP, tag="cc_out")
            nc.scalar.dma_start(cc_in[:], hp[:])
            nc.gpsimd.collective_compute(
                "AllGather",
                ALU.bypass,
                replica_groups=[list(range(NCORES))],
                ins=[cc_in[:]],
                outs=[cc_out[:]],
            )
            agt_a = sb.tile([NCORES, H // 2], FP, tag="agt_a")
            nc.scalar.dma_start(agt_a[:], cc_out[:, 0 : H // 2])
            agt_b = sb.tile([NCORES, H // 2], FP, tag="agt_b")
            nc.sync.dma_start(agt_b[:], cc_out[:, H // 2 : H])
            psRc = ps_tp.tile([P, KT], FP, tag="tp")
            for t in range(KT):
                half = agt_a if t < 4 else agt_b
                nc.tensor.matmul(
                    psRc[:, t : t + 1],
                    half[:, (t % 4) * P : (t % 4 + 1) * P],
                    ones_8[:],
                    start=True,
                    stop=True,
                )
            r_col = sb.tile([P, KT], FP, tag="r_col")
            nc.scalar.activation(r_col[:], psRc[:], AF.Relu)

            s_row = sb.tile([1, H], FP, tag="s_row")
            ps = ps_row.tile([1, RTOP], FP, tag="row")
            for k in range(KT):
                nc.tensor.matmul(
                    ps[:],
                    r_col[:, k : k + 1],
                    w3t_t[:, k, :],
                    start=(k == 0),
                    stop=False,
                )
            nc.tensor.matmul(
                ps[:], one_11[:], vt[0:1, 4 * H : 4 * H + RTOP], start=False, stop=True
            )
            nc.scalar.activation(s_row[0:1, 0:RTOP], ps[:], AF.Sigmoid, scale=-1.0)

            rrow2 = sb.tile([1, H], FP, tag="rrow2")
            nc.scalar.dma_start(rrow2[:], r_col[:])
            xb3 = sb.tile([P, H], FP, tag="xb3")
            nc.gpsimd.partition_broadcast(xb3[:], rrow2[0:1, :])
            zb = sb.tile([P, RKB], FP, tag="zb")
            for t in range(RKB):
                junk = jk.tile([P, H], FP, tag="jv")
                nc.vector.scalar_tensor_tensor(
                    junk[:], w3n_t[:, t, :], 1.0, xb3[:], op0=ALU.mult, op1=ALU.mult,
                    accum_out=zb[:, t : t + 1],
                )
            nc.vector.tensor_add(zb[:], zb[:], bg2_bot[:])
            psZ = ps_row.tile([1, RBOT], FP, tag="row")
            for t in range(RKB):
                nc.tensor.matmul(
                    psZ[0:1, t * P : (t + 1) * P],
                    zb[:, t : t + 1],
                    eye_t[:],
                    start=True,
                    stop=True,
                )
            nc.scalar.activation(s_row[0:1, RTOP:H], psZ[:], AF.Sigmoid, scale=-1.0)

            fin_a = sb.tile([1, RTOP], FP, tag="fin_a")
            nc.vector.tensor_mul(fin_a[:], outa_row[0:1, 0:RTOP], s_row[0:1, 0:RTOP])
            nc.sync.dma_start(fin.ap()[0:1, 0:RTOP], fin_a[:])
            fin_b = sb.tile([1, RBOT], FP, tag="fin_b")
            nc.vector.tensor_mul(fin_b[:], outa_row[0:1, RTOP:H], s_row[0:1, RTOP:H])
            nc.sync.dma_start(fin.ap()[0:1, RTOP:H], fin_b[:])

    nc.compile()
    return nc


def _make_in_maps_ref(inputs):
    Wexc = np.asarray(inputs["Wexc"], dtype=np.float32)
    Wg1 = np.asarray(inputs["Wg1"], dtype=np.float32)
    Wg2 = np.asarray(inputs["Wg2"], dtype=np.float32)
    blat = np.asarray(inputs["blat_e"], dtype=np.float32)
    bfb = np.asarray(inputs["bfb_e"], dtype=np.float32)
    bexc = np.asarray(inputs["bexc"], dtype=np.float32)
    bg1 = np.asarray(inputs["bg1"], dtype=np.float32)
    bg2 = np.asarray(inputs["bg2"], dtype=np.float32)
    tau = np.asarray(inputs["tau_exc"], dtype=np.float32)
    thr = np.asarray(inputs["threshold"], dtype=np.float32)

    s_idx = np.arange(H)
    rho = (s_idx % KT) * P + s_idx // KT
    eye = np.eye(P, dtype=np.float32)

    in_maps = []
    for c in range(NCORES):
        sl = slice(c * H, (c + 1) * H)
        srow = np.zeros((H,), np.float32)
        srow[0], srow[1] = tau[c], thr[c]
        bg2p = bg2[sl][rho]
        bg2_bot = bg2p[RTOP:].reshape(RKB, P).T.reshape(-1)
        row4 = np.concatenate([bg2p[:RTOP], bg2_bot])
        vecs = np.stack([blat[c, 0], bfb[c, 0], bexc[c, 0][rho], bg1[rho], row4, srow])
        w3pp = Wg2[sl][np.ix_(rho, rho)]
        in_maps.append(
            {
                "w1": np.ascontiguousarray(Wexc[c, 0]),
                "w2": np.ascontiguousarray(Wg1[:, sl][:, rho]),
                "w3t": np.ascontiguousarray(w3pp[0:RTOP, :].T),
                "w3n": np.ascontiguousarray(w3pp[RTOP:, :][:, rho]),
                "vecs": np.ascontiguousarray(vecs),
                "eye": eye,
            }
        )
    return in_maps


def _x0_is_zero(inputs):
    blat = np.asarray(inputs["blat_e"], dtype=np.float32)
    bfb = np.asarray(inputs["bfb_e"], dtype=np.float32)
    tau = np.asarray(inputs["tau_exc"], dtype=np.float32)
    thr = np.asarray(inputs["threshold"], dtype=np.float32)
    a = np.exp(-1.0 / tau)
    pre = (1.0 - a)[:, None] * (blat[:, 0] + bfb[:, 0]) - thr[:, None]
    return bool((pre <= 0.0).all())


def kernel(**inputs):
    if _x0_is_zero(inputs):
        bexc0 = np.asarray(inputs["bexc"], dtype=np.float32)[:, 0, :]
        bx_flat = np.maximum(bexc0, 0.0).reshape(-1)
        kgp, sel, vals = _prune_k(bx_flat)
        # output-row pruning (exact): final[i]=0 wherever out_c[i]==0
        nnz_c = [np.flatnonzero(bexc0[c] > 0.0) for c in range(NCORES)]
        kcp = max(TOP + P, int(np.ceil(max(s.size for s in nnz_c) / P)) * P)
        kcp = min(kcp, H)
        rsel = [
            np.concatenate([s, np.zeros(kcp - s.size, np.int64)])
            for s in nnz_c
        ]
        key = ("nc_fast", kgp, kcp)
        if key not in _CACHE:
            _CACHE[key] = _build_nc_fast(kgp, kcp)
        nc = _CACHE[key]
        in_maps = _make_in_maps_fast(inputs, kgp, sel, vals, kcp, rsel)
        # padded rows must multiply to zero in the final output
        for c in range(NCORES):
            m = in_maps[c]["osel"].copy()
            m[0, nnz_c[c].size :] = 0.0
            in_maps[c]["osel"] = m
        res = run_bass_kernel_spmd(nc, in_maps, core_ids=list(range(NCORES)))
        _CACHE["last_result"] = res
        out = np.zeros((NCORES, H), np.float32)
        for c in range(NCORES):
            packed = res.results[c]["final"].reshape(-1).astype(np.float32)
            ns = nnz_c[c].size
            out[c, nnz_c[c]] = packed[:ns]
        return out.reshape(-1)

    if "nc_ref" not in _CACHE:
        _CACHE["nc_ref"] = _build_nc_ref()
    nc = _CACHE["nc_ref"]
    in_maps = _make_in_maps_ref(inputs)
    res = run_bass_kernel_spmd(nc, in_maps, core_ids=list(range(NCORES)))
    _CACHE["last_result"] = res
    chunks = []
    for c in range(NCORES):
        st = res.results[c]["final"].reshape(P, KT)
        chunks.append(np.ascontiguousarray(st.T).reshape(-1))
    return np.concatenate(chunks).astype(np.float32)
